# revision 19
# baseline (speedup 1.0000x reference)
"""Trainium2 Bass kernel for nn_AttentionHead (B=4, S=2048, DK=1024).

Single-head attention with input projections:
    qp = q @ wq.T; kp = k @ wk.T; vp = v @ wv.T
    s  = qp @ kp.T / sqrt(dk); attn = softmax(s); out = attn @ vp

Sharding: 8 cores = (batch b in 0..3) x (sequence half h in 0..1).
Each core owns 1024 query rows AND 1024 key/value rows of its batch.
K/V projections are computed once per row (no duplication across the
pair): each core projects only its own 1024 k/v rows, then the pair
exchanges halves with 2-rank AllGathers through HBM bounce buffers
(SPMD-uniform layout: both halves are read back from the AllGather
output in global j-order, so the program is identical on all cores).

Per core (all matmul operands bf16, fp32 PSUM accumulation):
    A: kpT_loc[e,j'] = sum_d wkT[d,e] kT_loc[d,j']   (128 MMs)
       -> bounce -> AllGather(pair) -> kpT[e, 0:2048]
    E: vp_loc[j',e]  = sum_d vT_loc[d,j'] wvT[d,e]   (128 MMs)
       -> bounce -> AllGather(pair) -> vp[j] for all 16 j-tiles
    B: qpT[e,i]  = sum_d wqT[d,e] qT[d,i]            (128 MMs)
    C: sT[j,i]   = sum_e kpT[e,j] qpT[e,i]           (256 MMs)
       eT[j,i]   = exp(sT/32)  (ACT, fused scale, stays in SBUF)
       cs[i]     = sum_j eT[j,i] via ones-matmul      (32 MMs, trailing)
    F: outT[e,i] = (sum_j vp[j,e] eT[j,i]) * (1/cs[i])  (256 MMs)

DMA-descriptor generation is the hidden serial resource: each
dma_start costs ~0.6us of descriptor generation on its issuing
engine's sequencer, strictly in program order. The ~200 DMAs are
split across BOTH hardware DGE rings so neither backs up: input
streams + AllGather readbacks on the sync ring (consumption order),
bounce writes + output stores on the scalar ring. Collectives
trigger from gpsimd. Each ring's program order equals it
consumption order, so no head-of-line inversion is possible.

Phase-F accumulators come from the same rotating 6-bank PSUM pool
as the earlier phases (no pool close/open at the C->F boundary --
a fresh pool would wait on the colsum-reciprocal chain and stall
the PE ~6us). The reciprocal runs on the scalar engine.

928 matmuls/core at ~216 ns each (bf16; FWL + LDWEIGHTS prefetch
hide the weight-load bubble). Measured end-to-end relative error vs
the fp32 reference: ~6e-3 (bf16 quantization).
"""

import numpy as np

_B, _S, _DK = 4, 2048, 1024
_HALF = _S // 2
_N_CORES = 8
_P = 128
_PAIRS = [[0, 1], [2, 3], [4, 5], [6, 7]]

_CACHE = {}


def _emit(tc, qT, kTh, vTh, wqT, wkT, wvT, outT, cc, DK, S, HALF, mm_dt):
    import concourse.bass as bass
    from concourse import mybir

    nc = tc.nc
    ts = bass.ts
    P = _P
    NF = 512
    KH = S // 2            # local key/value rows
    DT = DK // P           # contraction tiles (d)
    ET = DK // P           # output-feature tiles (e)
    JT = S // P            # global key tiles (j)
    ISL = HALF // NF       # query slices (i)
    JSLH = KH // NF        # local key slices
    ESL = DK // NF         # feature slices
    JGN = KH // NF         # local vT chunk groups
    JPG = NF // P          # j-tiles per vT chunk
    NORM = 1.0 / float(np.sqrt(DK))
    f32 = mybir.dt.float32
    AF = mybir.ActivationFunctionType
    kp_in, kp_out, vp_in, vp_out = cc

    _cms = {}

    def opn(**kw):
        cm = tc.tile_pool(**kw)
        pool = cm.__enter__()
        _cms[id(pool)] = cm
        return pool

    def cls(*pools):
        for pool in pools:
            _cms.pop(id(pool)).__exit__(None, None, None)

    # ---------------- pools ----------------
    # LEFT stack: misc | x (stream rotation) | kpT | vp | qpT
    # RIGHT stack: stage | cst (bounce staging) | weights | eT
    misc = opn(name="misc", bufs=1, side="left")
    xp = opn(name="xp", bufs=1, side="left")
    stage = opn(name="stage", bufs=4, side="right")
    cstp = opn(name="cstp", bufs=1, side="right")
    wp = opn(name="wp", bufs=1, side="right")
    psmm = opn(name="psmm", bufs=6, space="PSUM")
    psacc = opn(name="psacc", bufs=1, space="PSUM")

    ones_f32 = misc.tile([P, P], f32, tag="ones_f32")
    nc.vector.memset(ones_f32[:], 1.0)
    ones = misc.tile([P, P], mm_dt, tag="ones")
    nc.vector.tensor_copy(ones[:], ones_f32[:])
    recip = misc.tile([P, HALF], f32, tag="recip")
    cs_ps = [psacc.tile([P, NF], f32, tag=f"cs{i}", name=f"cs{i}") for i in range(ISL)]

    # x-pool rotation: per-d stream chunks [P, NF], 2 slots.
    # Allocation order per d: k[0..1], vs[0..1]; each allocation is
    # emitted only after the slot's previous tenant has no more readers.
    # q rides the second buffer of the wv tag (no slot-wait: its transfers
    # finish before the AllGather readback floods hit the DMA engines).
    def x_tile(kind, d, idx):
        return xp.tile([P, NF], mm_dt, tag=f"x{d}", bufs=2, name=f"{kind}{idx}_d{d}")

    def cst_tile(nm):
        return cstp.tile([P, NF], mm_dt, tag="cst", bufs=12, name=nm)

    # ---------------- PE warm-up while first DMAs land ----------------
    warm_ps = psmm.tile([P, P], f32, tag="mm", name="warm_ps")
    for _ in range(64):
        nc.tensor.matmul(warm_ps[:], ones[:], ones[:], start=True, stop=True)

    # weights: wk in two [P,NF] chunks (first-chain latency), wv/wq as
    # single [P,DK] rows. Static tags, no slot rotation.
    EPC = NF // P
    wk_c = [[None] * 2 for _ in range(DT)]
    wv_c = [None] * DT
    wq_c = [None] * DT

    k_c = [[None] * JSLH for _ in range(DT)]
    vs_c = [[None] * JGN for _ in range(DT)]
    q_c = [None] * DT

    # sync-ring descriptor order == consumption order:
    # wk.h0, k0, wk.h1, k1, wv, vs0 | vs1 | wq, q0 | q1 (bars = emit points
    # constrained by x-slot reuse; see loop bodies below)
    for js in range(JSLH):
        for d in range(DT):
            k_c[d][js] = x_tile("k", d, js)
            nc.scalar.dma_start(k_c[d][js][:], kTh[ts(d, P), ts(js, NF)])
    for d in range(DT):
        t = wp.tile([P, NF], mm_dt, tag=f"wk{d}h0", name=f"wk{d}_0")
        nc.sync.dma_start(t[:], wkT[ts(d, P), ts(0, NF)])
        wk_c[d][0] = t
    for d in range(DT):
        t = wp.tile([P, NF], mm_dt, tag=f"wk{d}h1", name=f"wk{d}_1")
        nc.sync.dma_start(t[:], wkT[ts(d, P), ts(1, NF)])
        wk_c[d][1] = t
    for d in range(DT):
        t = wp.tile([P, DK], mm_dt, tag=f"wv{d}", bufs=2, name=f"wv{d}")
        nc.sync.dma_start(t[:], wvT[ts(d, P), :])
        wv_c[d] = t

    def wk_slice(d, e):
        return wk_c[d][e // EPC][:, ts(e % EPC, P)]

    # ---------------- phase A: local kpT half -> bounce -> AllGather ----
    kp_pool = opn(name="kpp", bufs=1, side="left")
    kpT = [kp_pool.tile([P, S], mm_dt, tag=f"kp{e}", name=f"kp{e}") for e in range(ET)]

    for js in range(JSLH):
        for e in range(ET):
            ps = psmm.tile([P, NF], f32, tag="mm")
            for d in range(DT):
                nc.tensor.matmul(
                    ps[:],
                    wk_slice(d, e),
                    k_c[d][js][:],
                    start=(d == 0),
                    stop=(d == DT - 1),
                )
            st = cst_tile(f"kpb{js}_{e}")
            nc.vector.tensor_copy(st[:], ps[:])
            nc.scalar.dma_start(kp_in[js][ts(e, P), :], st[:])
        # vs chunk js reuses the k slot freed by this js slab
        for d in range(DT):
            vs_c[d][js] = x_tile("vs", d, js)
            nc.sync.dma_start(vs_c[d][js][:], vTh[ts(d, P), ts(js, NF)])
        nc.gpsimd.collective_compute(
            "AllGather",
            mybir.AluOpType.bypass,
            replica_groups=_PAIRS,
            ins=[kp_in[js][:, :]],
            outs=[kp_out[js][:, :]],
        )

    # wq + q loads ride the sync ring behind the vs streams; q uses the
    # spare wv buffer so its transfers have no slot-wait and land before
    # the readback floods
    for d in range(DT):
        t = wp.tile([P, DK], mm_dt, tag=f"wq{d}", name=f"wq{d}")
        nc.sync.dma_start(t[:], wqT[ts(d, P), :])
        wq_c[d] = t
    for d in range(DT):
        t = wp.tile([P, DK], mm_dt, tag=f"wv{d}", bufs=2, name=f"q{d}")
        nc.sync.dma_start(t[:], qT[ts(d, P), :])
        q_c[d] = t

    # ---------------- phase E: local vp half -> bounce -> AllGather ----
    vp_pool = opn(name="vpp", bufs=1, side="left")
    vp = [vp_pool.tile([P, DK], mm_dt, tag=f"vp{j}", name=f"vp{j}") for j in range(JT)]

    for g in range(JGN):
        for jin in range(JPG):
            for es in range(ESL):
                ps = psmm.tile([P, NF], f32, tag="mm")
                for d in range(DT):
                    nc.tensor.matmul(
                        ps[:],
                        vs_c[d][g][:, ts(jin, P)],
                        wv_c[d][:, ts(es, NF)],
                        start=(d == 0),
                        stop=(d == DT - 1),
                    )
                st = cst_tile(f"vpb{g}_{jin}_{es}")
                nc.vector.tensor_copy(st[:], ps[:])
                nc.scalar.dma_start(
                    vp_in[ts(g * JPG + jin, P), ts(es, NF)], st[:]
                )
    nc.gpsimd.collective_compute(
        "AllGather",
        mybir.AluOpType.bypass,
        replica_groups=_PAIRS,
        ins=[vp_in[:, :]],
        outs=[vp_out[:, :]],
    )

    # kp readback on the sync ring after every input stream is queued
    # (these wait on the collectives; nothing consumption-critical queues
    # behind them). Both halves land in global j-order, in C's
    # consumption order (js0.hr0 j0-3, js1.hr0 j4-7, js0.hr1, js1.hr1).
    for hr in range(2):
        for js in range(JSLH):
            for e in range(ET):
                nc.sync.dma_start(
                    kpT[e][:, ts(hr * JSLH + js, NF)],
                    kp_out[js][ts(hr * ET + e, P), :],
                )

    # ---------------- phase B: qpT = (q @ wq.T).T ----------------
    qp_pool = opn(name="qpp", bufs=1, side="left")
    qpT = [
        qp_pool.tile([P, HALF], mm_dt, tag=f"qp{e}", name=f"qp{e}") for e in range(ET)
    ]
    for isl in range(ISL):
        for e in range(ET):
            ps = psmm.tile([P, NF], f32, tag="mm")
            for d in range(DT):
                nc.tensor.matmul(
                    ps[:],
                    wq_c[d][:, ts(e, P)],
                    q_c[d][:, ts(isl, NF)],
                    start=(d == 0),
                    stop=(d == DT - 1),
                )
            nc.vector.tensor_copy(qpT[e][:, ts(isl, NF)], ps[:])

    # vp readback (gated on the vp collective, done well before F)
    for hr in range(2):
        for jl in range(JT // 2):
            nc.sync.dma_start(
                vp[hr * (JT // 2) + jl][:, :], vp_out[ts(hr * (JT // 2) + jl, P), :]
            )

    # weights are dead after B; free their SBUF so eT can live there
    cls(wp)

    # ---------------- phase C: sT -> exp -> eT (SBUF) + trailing colsum ----
    et_pool = opn(name="etp", bufs=1, side="right")
    eT = [et_pool.tile([P, HALF], mm_dt, tag=f"et{j}", name=f"et{j}") for j in range(JT)]
    pending_cs = []
    for j in range(JT):
        for isl in range(ISL):
            ps = psmm.tile([P, NF], f32, tag="mm")
            for e in range(ET):
                nc.tensor.matmul(
                    ps[:],
                    kpT[e][:, ts(j, P)],
                    qpT[e][:, ts(isl, NF)],
                    start=(e == 0),
                    stop=(e == ET - 1),
                )
            nc.scalar.activation(eT[j][:, ts(isl, NF)], ps[:], AF.Exp, scale=NORM)
            pending_cs.append((j, isl))
            if len(pending_cs) > 1:
                pj, pisl = pending_cs.pop(0)
                nc.tensor.matmul(
                    cs_ps[pisl][:],
                    ones[:],
                    eT[pj][:, ts(pisl, NF)],
                    start=(pj == 0),
                    stop=(pj == JT - 1),
                )
    for pj, pisl in pending_cs:
        nc.tensor.matmul(
            cs_ps[pisl][:],
            ones[:],
            eT[pj][:, ts(pisl, NF)],
            start=(pj == 0),
            stop=(pj == JT - 1),
        )
    for isl in range(ISL):
        nc.vector.reciprocal(recip[:, ts(isl, NF)], cs_ps[isl][:])

    # ---------------- phase F: outT = (eT.T @ vp).T * recip ----------------
    # Accumulators come from the same rotating psmm pool (a fresh PSUM pool
    # would wait on the colsum/reciprocal chain before its first bank frees).
    OH = NF // 2  # output DMA split: halves the tail after the last chain
    for isl in range(ISL):
        for e in range(ET):
            pft = psmm.tile([P, NF], f32, tag="mm", name=f"pf{e}_{isl}")
            for j in range(JT):
                nc.tensor.matmul(
                    pft[:],
                    vp[j][:, ts(e, P)],
                    eT[j][:, ts(isl, NF)],
                    start=(j == 0),
                    stop=(j == JT - 1),
                )
            ot = stage.tile([P, NF], f32, tag="ost")
            nc.vector.tensor_mul(ot[:], pft[:], recip[:, ts(isl, NF)])
            for u in range(2):
                nc.sync.dma_start(
                    outT[ts(e, P), ts(isl * 2 + u, OH)],
                    ot[:, ts(u, OH)],
                )
    cls(qp_pool, vp_pool, kp_pool, xp, misc)
    cls(et_pool, cstp, stage)
    cls(psacc)
    cls(psmm)


def build_program(DK=_DK, S=_S, HALF=_HALF, mm_dtype="bfloat16"):
    """Build + compile the per-core Bass program. Returns the Bacc object."""
    import concourse.tile as tile
    from concourse import bacc, mybir

    f32 = mybir.dt.float32
    mm_dt = getattr(mybir.dt, mm_dtype)
    KH = S // 2
    NF = 512

    nc = bacc.Bacc(
        "TRN2",
        target_bir_lowering=False,
        debug=False,
        enable_asserts=False,
        num_devices=_N_CORES,
    )
    qT = nc.dram_tensor("qt", (DK, HALF), mm_dt, kind="ExternalInput").ap()
    kTh = nc.dram_tensor("kt", (DK, KH), mm_dt, kind="ExternalInput").ap()
    vTh = nc.dram_tensor("vt", (DK, KH), mm_dt, kind="ExternalInput").ap()
    wqT = nc.dram_tensor("wqt", (DK, DK), mm_dt, kind="ExternalInput").ap()
    wkT = nc.dram_tensor("wkt", (DK, DK), mm_dt, kind="ExternalInput").ap()
    wvT = nc.dram_tensor("wvt", (DK, DK), mm_dt, kind="ExternalInput").ap()
    outT = nc.dram_tensor("outt", (DK, HALF), f32, kind="ExternalOutput").ap()

    # HBM bounce buffers for the pair AllGathers (one per tensor: each
    # collective carries a ~25us firmware latency floor, so fewer is faster)
    kp_in = [
        nc.dram_tensor(f"kp_in{c}", (DK, NF), mm_dt, kind="Internal").ap()
        for c in range(KH // NF)
    ]
    kp_out = [
        nc.dram_tensor(f"kp_out{c}", (2 * DK, NF), mm_dt, kind="Internal").ap()
        for c in range(KH // NF)
    ]
    vp_in = nc.dram_tensor("vp_in", (KH, DK), mm_dt, kind="Internal").ap()
    vp_out = nc.dram_tensor("vp_out", (2 * KH, DK), mm_dt, kind="Internal").ap()

    with tile.TileContext(nc) as tc:
        _emit(
            tc,
            qT,
            kTh,
            vTh,
            wqT,
            wkT,
            wvT,
            outT,
            (kp_in, kp_out, vp_in, vp_out),
            DK,
            S,
            HALF,
            mm_dt,
        )
    nc.compile()
    return nc


def _in_maps(q, k, v, wq, wk, wv):
    """Shard full inputs into 8 per-core input maps (host-side transposes)."""
    import ml_dtypes

    bf16 = ml_dtypes.bfloat16
    wqT = np.ascontiguousarray(wq.T).astype(bf16)
    wkT = np.ascontiguousarray(wk.T).astype(bf16)
    wvT = np.ascontiguousarray(wv.T).astype(bf16)
    maps = []
    for c in range(_N_CORES):
        b, h = divmod(c, 2)
        sl = slice(h * _HALF, (h + 1) * _HALF)
        maps.append(
            {
                "qt": np.ascontiguousarray(q[b, sl, :].T).astype(bf16),
                "kt": np.ascontiguousarray(k[b, sl, :].T).astype(bf16),
                "vt": np.ascontiguousarray(v[b, sl, :].T).astype(bf16),
                "wqt": wqT,
                "wkt": wkT,
                "wvt": wvT,
            }
        )
    return maps


def kernel(q, k, v, wq, wk, wv):
    from concourse.bass_utils import run_bass_kernel_spmd

    q = np.asarray(q, np.float32)
    k = np.asarray(k, np.float32)
    v = np.asarray(v, np.float32)
    wq = np.asarray(wq, np.float32)
    wk = np.asarray(wk, np.float32)
    wv = np.asarray(wv, np.float32)

    if "nc" not in _CACHE:
        _CACHE["nc"] = build_program()
    nc = _CACHE["nc"]

    res = run_bass_kernel_spmd(
        nc, _in_maps(q, k, v, wq, wk, wv), core_ids=list(range(_N_CORES))
    )

    out = np.empty((_B, _S, _DK), np.float32)
    for c in range(_N_CORES):
        b, h = divmod(c, 2)
        out[b, h * _HALF : (h + 1) * _HALF, :] = res.results[c]["outt"].T
    return out
